# revision 1
# baseline (speedup 1.0000x reference)
"""Trainium2 Bass kernel for time-decayed causal KNN retrieval + fusion scoring.

Math (reference):
  sim_t[i,j] = cos(q_i, p_j) * exp(-l*|ti-tj|)
  masked     = causal(tj < ti) ? sim_t : -inf   (rows with no causal keep sim_t)
  top-7 by masked value -> cross-attn fusion -> deviation score  [Bq]

Device strategy (8 NeuronCores, pool-sharded):
  * For causal pairs exp(-l*|ti-tj|) = exp(-l*ti) * exp(l*tj): fold the decay
    (and the L2 norms) into the matmul operands on the host. Non-causal pairs
    get the wrong decay sign but are masked to -1e30 anyway.
  * Sort pool by time, shard it round-robin across the 8 cores (8192
    items/core, balanced); sort queries by time. The causal mask becomes a
    per-row column-prefix ci = searchsorted(shard_times, ti).
  * Each core: fp32r matmuls (full PE rate) of all 2048 queries against its
    shard, processing only the causal prefix of chunks per 128-query tile;
    chunks straddling the causal boundary get an additive iota>=ci -> -1e30
    mask; DVE max/max_index reduce each row's prefix to top-8 (value, index)
    candidates.
  * Host: merge 8x8 candidates/row, top-7, gather raw pool embeddings,
    softmax fusion + anomaly score (trivial FLOPs).
"""

import numpy as np

BQ, BN, H, K = 2048, 65536, 256, 7
NCORES = 8
LAMBDA = 0.1
GAMMA, DELTA = 0.5, 0.5
EPS = 1e-12
COS_EPS = 1e-8
CHUNK = 512
SHARD = BN // NCORES  # 8192
QTILE = 128
NTILES = BQ // QTILE  # 16
NEG = -1.0e30
NEG_THRESH = -1.0e29

_PROGRAM_CACHE = {}


def _build_program(pt_list, w0_list):
    import concourse.bacc as bacc
    import concourse.mybir as mybir
    import concourse.tile as tile

    f32 = mybir.dt.float32
    f32r = mybir.dt.float32r
    u32 = mybir.dt.uint32

    nc = bacc.Bacc("TRN2", target_bir_lowering=False, debug=False)

    qT_d = nc.dram_tensor("qT", [H, BQ], f32r, kind="ExternalInput")
    pT_d = nc.dram_tensor("pT", [H, SHARD], f32r, kind="ExternalInput")
    ci_d = nc.dram_tensor("ci", [NTILES, QTILE, 1], f32, kind="ExternalInput")
    iota_d = nc.dram_tensor("iota", [QTILE, CHUNK], f32, kind="ExternalInput")
    vals_d = nc.dram_tensor("vals", [BQ, 8], f32, kind="ExternalOutput")
    idx_d = nc.dram_tensor("idx", [BQ, 8], u32, kind="ExternalOutput")

    with tile.TileContext(nc) as tc:
        with (
            tc.tile_pool(name="resident", bufs=1) as resp,
            tc.tile_pool(name="band", bufs=2) as bandp,
            tc.tile_pool(name="small", bufs=4) as smallp,
            tc.tile_pool(name="outs", bufs=4) as outp,
            tc.tile_pool(name="psum", bufs=8, space="PSUM") as psump,
        ):
            p_sb = []
            q_sb = []
            for h in range(2):
                pt_tile = resp.tile([128, SHARD], f32r, tag=f"p{h}")
                nc.sync.dma_start(pt_tile[:], pT_d[h * 128 : (h + 1) * 128, :])
                p_sb.append(pt_tile)
                qt_tile = resp.tile([128, BQ], f32r, tag=f"q{h}")
                nc.sync.dma_start(qt_tile[:], qT_d[h * 128 : (h + 1) * 128, :])
                q_sb.append(qt_tile)
            iota_sb = resp.tile([QTILE, CHUNK], f32, tag="iota")
            nc.sync.dma_start(iota_sb[:], iota_d[:, :])

            for t in range(NTILES):
                pt_len = pt_list[t]
                w0 = w0_list[t]
                nchunks = pt_len // CHUNK
                band = bandp.tile([QTILE, pt_len], f32, tag="band")
                ci_t = smallp.tile([QTILE, 1], f32, tag="ci")
                nc.sync.dma_start(ci_t[:], ci_d[t, :, :])

                for c in range(nchunks):
                    ps = psump.tile([QTILE, CHUNK], f32, tag="ps")
                    for h in range(2):
                        nc.tensor.matmul(
                            ps[:],
                            q_sb[h][:, t * QTILE : (t + 1) * QTILE],
                            p_sb[h][:, c * CHUNK : (c + 1) * CHUNK],
                            start=(h == 0),
                            stop=(h == 1),
                        )
                    sl = slice(c * CHUNK, (c + 1) * CHUNK)
                    if (c + 1) * CHUNK <= w0:
                        # fully causal for every row in this tile
                        nc.scalar.copy(out=band[:, sl], in_=ps[:])
                    else:
                        # additive mask: -1e30 where global col >= ci(row)
                        thr = smallp.tile([QTILE, 1], f32, tag="thr")
                        nc.gpsimd.tensor_scalar(
                            thr[:],
                            ci_t[:],
                            float(c * CHUNK),
                            None,
                            op0=mybir.AluOpType.subtract,
                        )
                        m = smallp.tile([QTILE, CHUNK], f32, tag="m")
                        nc.gpsimd.tensor_scalar(
                            m[:],
                            iota_sb[:],
                            thr[:],
                            NEG,
                            op0=mybir.AluOpType.is_ge,
                            op1=mybir.AluOpType.mult,
                        )
                        nc.vector.tensor_add(band[:, sl], ps[:], m[:])

                v8 = outp.tile([QTILE, 8], f32, tag="v8")
                i8 = outp.tile([QTILE, 8], u32, tag="i8")
                nc.vector.max(out=v8[:], in_=band[:, :])
                nc.vector.max_index(out=i8[:], in_max=v8[:], in_values=band[:, :])
                nc.sync.dma_start(vals_d[t * QTILE : (t + 1) * QTILE, :], v8[:])
                nc.sync.dma_start(idx_d[t * QTILE : (t + 1) * QTILE, :], i8[:])

    nc.compile()
    return nc


def _prepare(query_emb, query_time, pool_emb, pool_time):
    """Host preprocessing: fold norms+decay into operands, sort, shard."""
    q = query_emb.astype(np.float64)
    p = pool_emb.astype(np.float64)
    qt = query_time.astype(np.float64)
    pt = pool_time.astype(np.float64)

    qnorm = np.linalg.norm(q, axis=1)
    pnorm = np.linalg.norm(p, axis=1)
    qs = (q / np.maximum(qnorm, EPS)[:, None]) * np.exp(-LAMBDA * qt)[:, None]
    ps = (p / np.maximum(pnorm, EPS)[:, None]) * np.exp(LAMBDA * pt)[:, None]

    pperm = np.argsort(pool_time, kind="stable")
    qperm = np.argsort(query_time, kind="stable")
    ps_sorted = ps[pperm]
    pt_sorted = pool_time[pperm]
    qs_sorted = qs[qperm]
    qt_sorted = query_time[qperm]

    shard_emb = [np.ascontiguousarray(ps_sorted[k::NCORES].T, dtype=np.float32) for k in range(NCORES)]
    shard_times = [pt_sorted[k::NCORES] for k in range(NCORES)]
    # exact count of shard items with tj < ti (strict), per core per sorted query
    ci = np.stack(
        [np.searchsorted(shard_times[k], qt_sorted, side="left") for k in range(NCORES)]
    ).astype(np.int64)  # [8, 2048]

    qT = np.ascontiguousarray(qs_sorted.T, dtype=np.float32)  # [256, 2048]
    return qT, shard_emb, ci, pperm, qperm


def _device_candidates(qT, shard_emb, ci):
    """Run the Bass kernel; return per-row merged candidate (vals, global sorted pos)."""
    from concourse.bass_utils import run_bass_kernel_spmd

    ci_tiles = ci.reshape(NCORES, NTILES, QTILE)
    maxci = ci_tiles.max(axis=0).max(axis=1)  # [NTILES]
    minci = ci_tiles.min(axis=0).min(axis=1)  # [NTILES]
    pt_list = np.clip(
        np.ceil(maxci / CHUNK).astype(np.int64) * CHUNK, CHUNK, SHARD
    ).tolist()
    w0_list = (np.floor(minci / CHUNK).astype(np.int64) * CHUNK).tolist()

    key = (tuple(pt_list), tuple(w0_list))
    if key not in _PROGRAM_CACHE:
        _PROGRAM_CACHE.clear()
        _PROGRAM_CACHE[key] = _build_program(pt_list, w0_list)
    nc = _PROGRAM_CACHE[key]

    iota = np.ascontiguousarray(
        np.broadcast_to(np.arange(CHUNK, dtype=np.float32), (QTILE, CHUNK))
    )
    in_maps = []
    for k in range(NCORES):
        in_maps.append(
            {
                "qT": qT,
                "pT": shard_emb[k],
                "ci": ci[k].astype(np.float32).reshape(NTILES, QTILE, 1),
                "iota": iota,
            }
        )
    res = run_bass_kernel_spmd(nc, in_maps, core_ids=list(range(NCORES)))
    vals = np.stack([res.results[k]["vals"] for k in range(NCORES)])  # [8, 2048, 8]
    idx = np.stack([res.results[k]["idx"] for k in range(NCORES)])  # [8, 2048, 8]
    return vals, idx


def _merge_and_score(
    vals, idx, pperm, qperm, query_emb, query_time, pool_emb, pool_time
):
    """Merge per-shard candidates, pick top-7, fuse and score (all rows)."""
    nq = BQ
    # candidate global position in time-sorted pool: local*NCORES + core
    cores = np.arange(NCORES)[:, None, None]
    gpos = idx.astype(np.int64) * NCORES + cores  # [8, 2048, 8]
    cvals = np.transpose(vals, (1, 0, 2)).reshape(nq, -1)  # [2048, 64]
    cpos = np.transpose(gpos, (1, 0, 2)).reshape(nq, -1)
    corig = pperm[cpos]  # original pool indices

    valid = cvals > NEG_THRESH
    # sort candidates per row: value desc, then original index asc (tie-break)
    order = np.lexsort((corig, -cvals.astype(np.float64)), axis=1)
    svals = np.take_along_axis(cvals, order, axis=1)
    sorig = np.take_along_axis(corig, order, axis=1)
    svalid = np.take_along_axis(valid, order, axis=1)

    top_idx = np.zeros((nq, K), dtype=np.int64)
    n_causal_global = np.searchsorted(np.sort(pool_time), query_time[qperm], side="left")

    pt_min = pool_time.min()
    for i in range(nq):
        oi = qperm[i]  # original query row
        ti = query_time[oi]
        if ti <= pt_min:
            # row_all_inf: reference keeps unmasked decayed sims; compute exactly
            qn = query_emb[oi] / max(np.linalg.norm(query_emb[oi]), EPS)
            pn = pool_emb / np.maximum(
                np.linalg.norm(pool_emb, axis=1), EPS
            )[:, None]
            sims = (pn @ qn) * np.exp(
                -LAMBDA * np.abs(ti - pool_time)
            )
            top_idx[i] = np.argsort(-sims, kind="stable")[:K]
            continue
        nvalid = int(svalid[i].sum())
        if nvalid >= K:
            top_idx[i] = sorig[i, :K]
        else:
            # pad like jax.lax.top_k over -inf ties: lowest original non-causal idx
            picks = list(sorig[i, :nvalid])
            need = K - nvalid
            j = 0
            while need > 0:
                if pool_time[j] >= ti and j not in picks:
                    picks.append(j)
                    need -= 1
                j += 1
            top_idx[i] = np.array(picks, dtype=np.int64)

    # fusion + score in float64 (reference is f32; fp64 is strictly closer)
    q = query_emb.astype(np.float64)[qperm]  # sorted-query order
    retrieved = pool_emb.astype(np.float64)[top_idx]  # [2048, 7, 256]
    scale = float(H) ** -0.5
    logits = np.einsum("bh,bkh->bk", q, retrieved) * scale
    logits -= logits.max(axis=1, keepdims=True)
    e = np.exp(logits)
    attn = e / e.sum(axis=1, keepdims=True)
    fused = np.einsum("bk,bkh->bh", attn, retrieved)

    qn2 = np.linalg.norm(q, axis=1)
    fn2 = np.linalg.norm(fused, axis=1)
    cos = np.sum(q * fused, axis=1) / np.maximum(qn2 * fn2, COS_EPS)
    l2 = np.linalg.norm(q - fused, axis=1)
    score_sorted = GAMMA * (1.0 - cos) + DELTA * l2

    out = np.zeros(nq, dtype=np.float32)
    out[qperm] = score_sorted.astype(np.float32)
    return out


def kernel(query_emb, query_time, pool_emb, pool_time):
    query_emb = np.asarray(query_emb, dtype=np.float32)
    query_time = np.asarray(query_time, dtype=np.float32)
    pool_emb = np.asarray(pool_emb, dtype=np.float32)
    pool_time = np.asarray(pool_time, dtype=np.float32)

    qT, shard_emb, ci, pperm, qperm = _prepare(
        query_emb, query_time, pool_emb, pool_time
    )
    vals, idx = _device_candidates(qT, shard_emb, ci)
    return _merge_and_score(
        vals, idx, pperm, qperm, query_emb, query_time, pool_emb, pool_time
    )


# revision 12
# speedup vs baseline: 46.7442x; 46.7442x over previous
"""Trainium2 Bass kernel for time-decayed causal KNN retrieval + fusion scoring.

Math (reference):
  sim_t[i,j] = cos(q_i, p_j) * exp(-l*|ti-tj|)
  masked     = causal(tj < ti) ? sim_t : -inf   (rows with no causal keep sim_t)
  top-7 by masked value -> cross-attn fusion -> deviation score  [Bq]

Strategy (8 NeuronCores, pool-sharded):
  * For causal pairs exp(-l*|ti-tj|) = exp(-l*ti)*exp(l*tj): fold the decay
    and the L2 norms into the matmul operands on the host (non-causal pairs
    get a wrong decay but are masked out on the host anyway).
  * Sort pool by time, shard round-robin across 8 cores (8192 items/core,
    balanced); sort queries by time. The causal mask becomes a column-prefix
    per row; only the causal prefix of 512-col chunks is computed per
    128-query tile (~45% of the full slab skipped).
  * Device per core: fp32r matmuls (full PE rate) -> PSUM; ACT stages the
    even/odd column halves to SBUF as bf16; DVE runs a 3-level pairwise-max
    tree (bf16 2x mode) producing per-row maxima of 8-element windows; the
    window-max band (<=1024 windows/tile) is DMA'd out. No top-k on device.
  * Host: applies the exact causal window kill, takes each row's global
    top-7-by-window-max threshold (bf16 monotonicity makes the containment
    of the true top-7 elements exact), rescores the selected ~10 windows'
    80 columns exactly in float64, selects top-7 with reference tie
    semantics, and computes the softmax fusion + anomaly score (trivial
    FLOPs).

Window layout (64-col blocks, contiguous APs at every tree level):
  level1 pairs adjacent cols (2j, 2j+1); level2 pairs j, j+16 inside each
  32-wide half-block; level3 pairs j, j+8. Window w covers columns
  64*(w//8) + 2*(w%8) + {0,1,16,17,32,33,48,49}; its minimum column is
  mincol(w) = 64*(w//8) + 2*(w%8), independent of the chunk grouping.
"""

import numpy as np

BQ, BN, H, K = 2048, 65536, 256, 7
NCORES = 8
LAMBDA = 0.1
GAMMA, DELTA = 0.5, 0.5
EPS = 1e-12
COS_EPS = 1e-8
CHUNK = 512
SHARD = BN // NCORES  # 8192
QTILE = 128
NTILES = BQ // QTILE  # 16
NWIN_MAX = SHARD // 8  # 1024
WIN_OFFS = np.array([0, 1, 16, 17, 32, 33, 48, 49], dtype=np.int64)
MAXW_ROW = 24  # cap on host-selected windows per row before full fallback

_PROGRAM_CACHE = {}


def _mincol():
    w = np.arange(NWIN_MAX, dtype=np.int64)
    return 64 * (w // 8) + 2 * (w % 8)


def _build_program(pt_list, reps=1):
    import concourse.bacc as bacc
    import concourse.mybir as mybir
    import concourse.tile as tile

    f32r = mybir.dt.float32r
    f32 = mybir.dt.float32
    bf16 = mybir.dt.bfloat16

    nc = bacc.Bacc("TRN2", target_bir_lowering=False, debug=False)

    qT_d = nc.dram_tensor("qT", [H, BQ], f32r, kind="ExternalInput")
    pT_d = nc.dram_tensor("pT", [H, SHARD], f32r, kind="ExternalInput")
    wb_d = nc.dram_tensor("wb", [BQ, NWIN_MAX], bf16, kind="ExternalOutput")

    GRP = 4  # chunks per PSUM tile (4 banks); 2 tiles in flight = 8 banks
    GW = GRP * CHUNK
    MAXOP = mybir.AluOpType.max

    with tile.TileContext(nc) as tc:
        with (
            tc.tile_pool(name="resident", bufs=1) as resp,
            tc.tile_pool(name="wband", bufs=2) as wbandp,
            tc.tile_pool(name="lvl", bufs=3) as lvlp,
            tc.tile_pool(name="psum", bufs=2, space="PSUM") as psump,
        ):
          for _rep in range(reps):
            p_sb = []
            q_sb = []
            for h in range(2):
                pt_tile = resp.tile([128, SHARD], f32r, tag=f"p{h}")
                nc.sync.dma_start(pt_tile[:], pT_d[h * 128 : (h + 1) * 128, :])
                p_sb.append(pt_tile)
                qt_tile = resp.tile([128, BQ], f32r, tag=f"q{h}")
                nc.sync.dma_start(qt_tile[:], qT_d[h * 128 : (h + 1) * 128, :])
                q_sb.append(qt_tile)

            for t in range(NTILES):
                pt_len = pt_list[t]
                nchunks = pt_len // CHUNK
                nwin = pt_len // 8
                wband = wbandp.tile([QTILE, nwin], bf16, tag="wband")

                for g in range(0, nchunks, GRP):
                    ge = min(g + GRP, nchunks)
                    gw = (ge - g) * CHUNK
                    ps = psump.tile([QTILE, gw], f32, tag="ps", name=f"ps{t}_{g}")
                    # h-outer: load each stationary q-tile once per group
                    for h in range(2):
                        for j, c in enumerate(range(g, ge)):
                            nc.tensor.matmul(
                                ps[:, j * CHUNK : (j + 1) * CHUNK],
                                q_sb[h][:, t * QTILE : (t + 1) * QTILE],
                                p_sb[h][:, c * CHUNK : (c + 1) * CHUNK],
                                start=(h == 0),
                                stop=(h == 1),
                                skip_group_check=True,
                            )
                    # stage even/odd column halves to SBUF as bf16 (ACT)
                    w1a = lvlp.tile([QTILE, GW // 2], bf16, tag="w1a")
                    w1b = lvlp.tile([QTILE, GW // 2], bf16, tag="w1b")
                    nc.scalar.copy(out=w1a[:, : gw // 2], in_=ps[:, 0:gw:2])
                    nc.scalar.copy(out=w1b[:, : gw // 2], in_=ps[:, 1:gw:2])
                    # 3-level pairwise max tree on DVE, all-contiguous bf16
                    w1 = lvlp.tile([QTILE, GW // 2], bf16, tag="w1")
                    nc.vector.tensor_tensor(
                        out=w1[:, : gw // 2],
                        in0=w1a[:, : gw // 2],
                        in1=w1b[:, : gw // 2],
                        op=MAXOP,
                    )
                    w1r = w1[:, : gw // 2].rearrange("p (b x) -> p b x", x=32)
                    w2 = lvlp.tile([QTILE, GW // 4], bf16, tag="w2")
                    w2r = w2[:, : gw // 4].rearrange("p (b x) -> p b x", x=16)
                    nc.vector.tensor_tensor(
                        out=w2r[:, :, :],
                        in0=w1r[:, :, 0:16],
                        in1=w1r[:, :, 16:32],
                        op=MAXOP,
                    )
                    wbr = wband[
                        :, g * (CHUNK // 8) : g * (CHUNK // 8) + gw // 8
                    ].rearrange("p (b x) -> p b x", x=8)
                    nc.vector.tensor_tensor(
                        out=wbr[:, :, :],
                        in0=w2r[:, :, 0:8],
                        in1=w2r[:, :, 8:16],
                        op=MAXOP,
                    )

                nc.sync.dma_start(
                    wb_d[t * QTILE : (t + 1) * QTILE, :nwin], wband[:]
                )

    nc.compile()
    return nc


def _prepare(query_emb, query_time, pool_emb, pool_time):
    """Host preprocessing: fold norms+decay into operands, sort, shard."""
    q = query_emb.astype(np.float64)
    p = pool_emb.astype(np.float64)
    qt = query_time.astype(np.float64)
    pt = pool_time.astype(np.float64)

    qnorm = np.linalg.norm(q, axis=1)
    pnorm = np.linalg.norm(p, axis=1)
    qs = (q / np.maximum(qnorm, EPS)[:, None]) * np.exp(-LAMBDA * qt)[:, None]
    ps = (p / np.maximum(pnorm, EPS)[:, None]) * np.exp(LAMBDA * pt)[:, None]

    pperm = np.argsort(pool_time, kind="stable")
    qperm = np.argsort(query_time, kind="stable")
    ps_sorted = ps[pperm]
    pt_sorted = pool_time[pperm]
    qs_sorted = qs[qperm]
    qt_sorted = query_time[qperm]

    shard_emb = [
        np.ascontiguousarray(ps_sorted[k::NCORES].T, dtype=np.float32)
        for k in range(NCORES)
    ]
    shard_times = [pt_sorted[k::NCORES] for k in range(NCORES)]
    # exact count of shard items with tj < ti (strict), per core per sorted query
    ci = np.stack(
        [np.searchsorted(shard_times[k], qt_sorted, side="left") for k in range(NCORES)]
    ).astype(np.int64)  # [8, 2048]

    qT = np.ascontiguousarray(qs_sorted.T, dtype=np.float32)  # [256, 2048]
    return qT, shard_emb, ci, pperm, qperm


def _pt_list(ci):
    ci_tiles = ci.reshape(NCORES, NTILES, QTILE)
    maxci = ci_tiles.max(axis=0).max(axis=1)  # [NTILES]
    return np.clip(
        np.ceil(maxci / CHUNK).astype(np.int64) * CHUNK, CHUNK, SHARD
    ).tolist()


def _core_in_map(qT, shard_emb, k):
    return {"qT": qT, "pT": shard_emb[k]}


def _device_windows(qT, shard_emb, ci):
    """Run the Bass kernel; return per-core window-max bands [8, 2048, 1024]."""
    from concourse.bass_utils import run_bass_kernel_spmd

    pt_list = _pt_list(ci)
    key = tuple(pt_list)
    if key not in _PROGRAM_CACHE:
        _PROGRAM_CACHE.clear()
        _PROGRAM_CACHE[key] = _build_program(pt_list)
    nc = _PROGRAM_CACHE[key]

    in_maps = [_core_in_map(qT, shard_emb, k) for k in range(NCORES)]
    res = run_bass_kernel_spmd(nc, in_maps, core_ids=list(range(NCORES)))
    wb = np.stack(
        [res.results[k]["wb"].astype(np.float32) for k in range(NCORES)]
    )  # [8, 2048, 1024]
    return wb, pt_list


def _merge_and_score(
    wb, pt_list, ci, pperm, qperm, query_emb, query_time, pool_emb, pool_time
):
    """Select candidate windows by global threshold, rescore exactly, score."""
    nq = BQ
    mincol = _mincol()  # [1024]

    # validity: window exists for the row's tile and contains >=1 causal col
    nwin_row = (np.asarray(pt_list, dtype=np.int64) // 8)[
        np.repeat(np.arange(NTILES), QTILE)
    ]  # [2048]
    exists = np.arange(NWIN_MAX)[None, :] < nwin_row[:, None]  # [2048, 1024]
    wbm = np.where(
        exists[None, :, :] & (mincol[None, None, :] < ci[:, :, None]),
        wb,
        -np.inf,
    )  # [8, 2048, 1024]

    flat = np.transpose(wbm, (1, 0, 2)).reshape(nq, NCORES * NWIN_MAX)
    kth = np.partition(flat, -K, axis=1)[:, -K]  # 7th largest per row
    thr = np.where(np.isfinite(kth), kth, np.float32(np.inf))
    sel = flat >= thr[:, None]  # includes bf16 ties; exact containment
    nsel = sel.sum(axis=1)

    rows, wcols = np.nonzero(sel)
    core = wcols // NWIN_MAX
    w = wcols % NWIN_MAX
    # candidate columns: global time-sorted position -> original pool index
    cols_shard = mincol[w][:, None] + WIN_OFFS[None, :]  # [nsel, 8]
    sorted_pos = cols_shard * NCORES + core[:, None]
    orig = pperm[sorted_pos]  # [nsel_total, 8] original pool rows

    # exact rescore in float64
    q64 = query_emb.astype(np.float64)
    qn64 = q64 / np.maximum(np.linalg.norm(q64, axis=1), EPS)[:, None]
    pnorm = np.linalg.norm(pool_emb.astype(np.float64), axis=1)
    oi_rows = qperm[rows]  # original query row per selected window
    n_ent = rows.shape[0]
    sims = np.empty((n_ent, 8), dtype=np.float64)
    causal = np.empty((n_ent, 8), dtype=bool)
    BLK = 65536
    for b in range(0, n_ent, BLK):
        sl = slice(b, b + BLK)
        emb = pool_emb[orig[sl]].astype(np.float64)  # [blk, 8, 256]
        pn = np.maximum(pnorm[orig[sl]], EPS)
        dots = np.einsum("nh,nch->nc", qn64[oi_rows[sl]], emb) / pn
        tdiff = np.abs(
            query_time[oi_rows[sl]].astype(np.float64)[:, None]
            - pool_time[orig[sl]].astype(np.float64)
        )
        sims[sl] = dots * np.exp(-LAMBDA * tdiff)
        causal[sl] = pool_time[orig[sl]] < query_time[oi_rows[sl]][:, None]

    # scatter into dense per-row candidate arrays
    maxw = min(int(nsel.max()), MAXW_ROW)
    slot = np.zeros(n_ent, dtype=np.int64)
    if n_ent:
        # rows is sorted; position of each entry within its row
        row_start = np.searchsorted(rows, np.arange(nq), side="left")
        slot = np.arange(n_ent) - row_start[rows]
    keep = slot < MAXW_ROW
    dsims = np.full((nq, maxw * 8), -np.inf)
    dorig = np.zeros((nq, maxw * 8), dtype=np.int64)
    rk = rows[keep]
    sk = slot[keep]
    for o in range(8):
        dsims[rk, sk * 8 + o] = np.where(causal[keep, o], sims[keep, o], -np.inf)
        dorig[rk, sk * 8 + o] = orig[keep, o]

    order2 = np.lexsort((dorig, -dsims), axis=1)[:, :K]
    top_idx = np.take_along_axis(dorig, order2, axis=1)
    nvalid_row = np.isfinite(np.take_along_axis(dsims, order2, axis=1)).sum(axis=1)

    # rows needing the exact slow path
    pt_min = pool_time.min()
    n_causal_global = np.searchsorted(
        np.sort(pool_time), query_time[qperm], side="left"
    )
    fix_rows = np.nonzero(
        (query_time[qperm] <= pt_min)
        | (np.minimum(n_causal_global, K) > nvalid_row)
        | (n_causal_global < K)
        | (nsel > MAXW_ROW)
    )[0]
    if len(fix_rows):
        pn_all = pool_emb.astype(np.float64) / np.maximum(pnorm, EPS)[:, None]
    for i in fix_rows:
        oi = qperm[i]
        ti = query_time[oi]
        sims_all = (pn_all @ qn64[oi]) * np.exp(
            -LAMBDA * np.abs(float(ti) - pool_time.astype(np.float64))
        )
        if ti <= pt_min:
            # row_all_inf: reference keeps unmasked decayed sims
            top_idx[i] = np.argsort(-sims_all, kind="stable")[:K]
            continue
        causal_all = pool_time < ti
        c = int(causal_all.sum())
        masked_all = np.where(causal_all, sims_all, -np.inf)
        picks = list(np.argsort(-masked_all, kind="stable")[: min(c, K)])
        # pad like jax.lax.top_k over -inf ties: lowest non-causal original idx
        j = 0
        while len(picks) < K:
            if not causal_all[j]:
                picks.append(j)
            j += 1
        top_idx[i] = np.array(picks, dtype=np.int64)

    # fusion + score in float64 (reference is f32; fp64 is strictly closer)
    q = query_emb.astype(np.float64)[qperm]  # sorted-query order
    retrieved = pool_emb.astype(np.float64)[top_idx]  # [2048, 7, 256]
    scale = float(H) ** -0.5
    logits = np.einsum("bh,bkh->bk", q, retrieved) * scale
    logits -= logits.max(axis=1, keepdims=True)
    e = np.exp(logits)
    attn = e / e.sum(axis=1, keepdims=True)
    fused = np.einsum("bk,bkh->bh", attn, retrieved)

    qn2 = np.linalg.norm(q, axis=1)
    fn2 = np.linalg.norm(fused, axis=1)
    cos = np.sum(q * fused, axis=1) / np.maximum(qn2 * fn2, COS_EPS)
    l2 = np.linalg.norm(q - fused, axis=1)
    score_sorted = GAMMA * (1.0 - cos) + DELTA * l2

    out = np.zeros(nq, dtype=np.float32)
    out[qperm] = score_sorted.astype(np.float32)
    return out


def kernel(query_emb, query_time, pool_emb, pool_time):
    query_emb = np.asarray(query_emb, dtype=np.float32)
    query_time = np.asarray(query_time, dtype=np.float32)
    pool_emb = np.asarray(pool_emb, dtype=np.float32)
    pool_time = np.asarray(pool_time, dtype=np.float32)

    qT, shard_emb, ci, pperm, qperm = _prepare(
        query_emb, query_time, pool_emb, pool_time
    )
    wb, pt_list = _device_windows(qT, shard_emb, ci)
    return _merge_and_score(
        wb, pt_list, ci, pperm, qperm, query_emb, query_time, pool_emb, pool_time
    )
